# revision 4
# baseline (speedup 1.0000x reference)
"""Causal attention (B=2, L=2048, H=8, E=64) returning (V, SA) on 8 trn2 NeuronCores.

Sharding: the 16 (b,h) pairs are split 2-per-core (batch*head data parallel);
each core holds full L so the causal softmax needs no communication.

Per (b,h) on-chip plan (all matmuls bf16, accumulation f32):
  prep:   load Q,K,V f32 -> cast bf16; build Q^T,K^T [64,2048] via
          matmul-by-identity (PE) + PSUM->SBUF copy; V plus a ones column.
  phase1: S = Q K^T per 128-row tile -> additive causal mask on the diagonal
          block -> ACT exp(0.125*x) -> unnormalized E (bf16, SBUF).
  phase2: same matmuls with lhsT/rhs swapped -> E^T (bf16) directly
          (avoids any on-chip transpose of the 2048x2048 matrix).
  phase3: O_unnorm = E^T-blocks @ [V | 1]: the ones column accumulates the
          softmax row sums in PSUM col 64. reciprocal -> normalize SA rows
          (bf16*recip -> f32) and O, DMA out.
The strictly-upper causal triangle of SA is never written: PJRT donates
zero-initialized output buffers, so those regions read back as exact 0.
"""

import sys
import types

sys.path.insert(0, "/opt/trn_rl_repo")

import numpy as np
import ml_dtypes

B, L, H, E = 2, 2048, 8, 64
P = 128          # partitions / l-tile size
T = L // P       # 16 tiles
NBH = 2          # (b,h) pairs per core
N_CORES = 8
SCALE = 1.0 / 8.0
NEG = -1.0e9

_CACHE = {}


def _install_ntff_hook():
    """The image's antenv lacks axon_hooks; shim it so trace=True works."""
    import antenv

    if "antenv.axon_hooks" in sys.modules:
        return
    hooks = types.ModuleType("antenv.axon_hooks")
    hooks._hook = None
    hooks.set_axon_ntff_profile_hook = lambda h: setattr(hooks, "_hook", h)
    hooks.get_axon_ntff_profile_hook = lambda: hooks._hook
    sys.modules["antenv.axon_hooks"] = hooks
    antenv.axon_hooks = hooks
    try:
        from trn_agent_boot.trn_boot import _ntff_profile_via_ctypes

        hook = _ntff_profile_via_ctypes("/opt/axon/libaxon_pjrt.so")
        if hook is not None:
            hooks.set_axon_ntff_profile_hook(hook)
    except Exception:
        pass


def _emit(nc, tc, ctx, aps, mybir):
    import concourse.tile as tile  # noqa: F401

    f32 = mybir.dt.float32
    bf16 = mybir.dt.bfloat16
    Exp = mybir.ActivationFunctionType.Exp

    q = aps["q"].rearrange("n (t p) e -> n p t e", p=P)
    k = aps["k"].rearrange("n (t p) e -> n p t e", p=P)
    v = aps["v"].rearrange("n (t p) e -> n p t e", p=P)
    sa = aps["sa"].rearrange("n (t p) s -> n t p s", p=P)
    o = aps["o"].rearrange("n (t p) e -> n t p e", p=P)

    # packed-causal column offsets inside the E / E^T sbuf tiles
    offE = [P * (i * (i + 1) // 2) for i in range(T + 1)]            # row-tile i: width (i+1)*P
    offT = [P * (T * j - j * (j - 1) // 2) for j in range(T + 1)]    # col-tile j: width (T-j)*P
    CW = offE[T]  # 17408

    cpool = ctx.enter_context(tc.tile_pool(name="consts", bufs=1))
    ldpool = ctx.enter_context(tc.tile_pool(name="ld", bufs=3))
    bfpool = ctx.enter_context(tc.tile_pool(name="bfp", bufs=2))
    qktpool = ctx.enter_context(tc.tile_pool(name="qkt", bufs=2))
    bigpool = ctx.enter_context(tc.tile_pool(name="big", bufs=1))
    outpool = ctx.enter_context(tc.tile_pool(name="outp", bufs=3))
    smallpool = ctx.enter_context(tc.tile_pool(name="small", bufs=4))
    pspool = ctx.enter_context(tc.tile_pool(name="ps", bufs=2, space="PSUM"))

    mask_sb = cpool.tile([P, P], f32, tag="mask")
    nc.sync.dma_start(mask_sb[:], aps["maskneg"][:])
    maskT_sb = cpool.tile([P, P], f32, tag="maskT")
    nc.sync.dma_start(maskT_sb[:], aps["masknegT"][:])
    ident_sb = cpool.tile([P, P], bf16, tag="ident")
    nc.sync.dma_start(ident_sb[:], aps["ident"][:])

    for bh in range(NBH):
        # ---- load + cast ----
        qf = ldpool.tile([P, T, E], f32, tag="ld")
        nc.sync.dma_start(qf[:], q[bh])
        kf = ldpool.tile([P, T, E], f32, tag="ld")
        nc.sync.dma_start(kf[:], k[bh])
        vf = ldpool.tile([P, T, E], f32, tag="ld")
        nc.sync.dma_start(vf[:], v[bh])

        qb = bfpool.tile([P, T, E], bf16, tag="qb")
        nc.vector.tensor_copy(qb[:], qf[:])
        kb = bfpool.tile([P, T, E], bf16, tag="kb")
        nc.vector.tensor_copy(kb[:], kf[:])
        vones = bfpool.tile([P, T, E + 1], bf16, tag="vones")
        nc.vector.tensor_copy(vones[:, :, 0:E], vf[:])
        nc.vector.memset(vones[:, :, E : E + 1], 1.0)

        # ---- Q^T / K^T via matmul-by-identity (one batched PSUM->SBUF cast) ----
        qt = qktpool.tile([E, L], bf16, tag="qt")
        kt = qktpool.tile([E, L], bf16, tag="kt")
        for src, dst in ((qb, qt), (kb, kt)):
            pt = pspool.tile([E, L], f32, tag="ps")
            for t in range(T):
                nc.tensor.matmul(
                    pt[:, t * P : (t + 1) * P], src[:, t, :], ident_sb[:],
                    start=True, stop=True,
                )
            nc.vector.tensor_copy(dst[:], pt[:])

        # ---- phase 1: E = exp(scale * (Q K^T + causal mask)), row-tile layout ----
        e_sb = bigpool.tile([P, CW], bf16, tag="e")
        for i in range(T):
            W = (i + 1) * P
            ps = pspool.tile([P, 2048], f32, tag="ps")
            for c0 in range(0, W, 512):
                w = min(512, W - c0)
                nc.tensor.matmul(
                    ps[:, c0 : c0 + w],
                    qt[:, i * P : (i + 1) * P],
                    kt[:, c0 : c0 + w],
                    start=True,
                    stop=True,
                )
            nc.vector.tensor_add(ps[:, i * P : W], ps[:, i * P : W], mask_sb[:])
            nc.scalar.activation(
                e_sb[:, offE[i] : offE[i] + W], ps[:, 0:W], Exp, scale=SCALE
            )

        # ---- phase 2: E^T directly (swap lhsT/rhs), col-tile layout ----
        et_sb = bigpool.tile([P, CW], bf16, tag="et")
        for j in range(T):
            Wj = (T - j) * P
            base = j * P
            ps = pspool.tile([P, 2048], f32, tag="ps")
            for c0 in range(0, Wj, 512):
                w = min(512, Wj - c0)
                nc.tensor.matmul(
                    ps[:, c0 : c0 + w],
                    kt[:, base : base + P],
                    qt[:, base + c0 : base + c0 + w],
                    start=True,
                    stop=True,
                )
            nc.vector.tensor_add(ps[:, 0:P], ps[:, 0:P], maskT_sb[:])
            nc.scalar.activation(
                et_sb[:, offT[j] : offT[j] + Wj], ps[:, 0:Wj], Exp, scale=SCALE
            )

        # ---- phase 3: O^T = [V|1]^T @ E^T-chunks (V stationary, 512-wide moving),
        # then transpose back per l-tile, normalize, store ----
        CH = 512
        for c in range(L // CH):  # 4 chunks of 512 l-columns
            pot = pspool.tile([E + 1, CH], f32, tag="ps")
            jmax = 4 * c + 3
            for j in range(jmax + 1):
                lo = max(CH * c, j * P)  # columns below j*P are causal zeros
                n = CH * (c + 1) - lo
                nc.tensor.matmul(
                    pot[:, lo - CH * c : CH],
                    vones[:, j, :],
                    et_sb[:, offT[j] + lo - j * P : offT[j] + lo - j * P + n],
                    start=(j == 0),
                    stop=(j == jmax),
                )
            ot_sb = outpool.tile([E + 1, CH], bf16, tag="ot")
            nc.vector.tensor_copy(ot_sb[:], pot[:])

            for ii in range(CH // P):  # transpose back per l-tile
                i = 4 * c + ii
                W = (i + 1) * P
                ptb = pspool.tile([P, E + 1], f32, tag="ps")
                nc.tensor.matmul(
                    ptb[:],
                    ot_sb[:, ii * P : (ii + 1) * P],
                    ident_sb[0 : E + 1, 0 : E + 1],
                    start=True,
                    stop=True,
                )
                recip = smallpool.tile([P, 1], f32, tag="recip")
                nc.vector.reciprocal(recip[:], ptb[:, E : E + 1])

                sa_st = outpool.tile([P, 2048], f32, tag="sa")
                nc.vector.tensor_scalar_mul(
                    sa_st[:, 0:W], e_sb[:, offE[i] : offE[i] + W], recip[:]
                )
                nc.sync.dma_start(sa[bh, i, :, 0:W], sa_st[:, 0:W])

                o_st = outpool.tile([P, E], f32, tag="o")
                nc.vector.tensor_scalar_mul(o_st[:], ptb[:, 0:E], recip[:])
                nc.sync.dma_start(o[bh, i], o_st[:])


def build_nc():
    if "nc" in _CACHE:
        return _CACHE["nc"]
    from contextlib import ExitStack

    import concourse.bacc as bacc
    import concourse.mybir as mybir
    import concourse.tile as tile

    f32 = mybir.dt.float32
    bf16 = mybir.dt.bfloat16

    nc = bacc.Bacc("TRN2", target_bir_lowering=False, debug=False, num_devices=N_CORES)
    aps = {
        "q": nc.dram_tensor("q", [NBH, L, E], f32, kind="ExternalInput").ap(),
        "k": nc.dram_tensor("k", [NBH, L, E], f32, kind="ExternalInput").ap(),
        "v": nc.dram_tensor("v", [NBH, L, E], f32, kind="ExternalInput").ap(),
        "maskneg": nc.dram_tensor("maskneg", [P, P], f32, kind="ExternalInput").ap(),
        "masknegT": nc.dram_tensor("masknegT", [P, P], f32, kind="ExternalInput").ap(),
        "ident": nc.dram_tensor("ident", [P, P], bf16, kind="ExternalInput").ap(),
        "sa": nc.dram_tensor("sa", [NBH, L, L], f32, kind="ExternalOutput").ap(),
        "o": nc.dram_tensor("o", [NBH, L, E], f32, kind="ExternalOutput").ap(),
    }
    with tile.TileContext(nc) as tc, ExitStack() as ctx:
        _emit(nc, tc, ctx, aps, mybir)
    nc.compile()
    _CACHE["nc"] = nc
    return nc


def _host_consts():
    idx = np.arange(P)
    maskneg = np.where(idx[None, :] <= idx[:, None], 0.0, NEG).astype(np.float32)
    masknegT = maskneg.T.copy()
    ident = np.eye(P, dtype=ml_dtypes.bfloat16)
    return maskneg, masknegT, ident


def make_in_maps(queries, keys, values):
    queries = np.asarray(queries, dtype=np.float32)
    keys = np.asarray(keys, dtype=np.float32)
    values = np.asarray(values, dtype=np.float32)
    maskneg, masknegT, ident = _host_consts()
    in_maps = []
    for c in range(N_CORES):
        pairs = [2 * c, 2 * c + 1]
        qs = np.stack([queries[m // H, :, m % H, :] for m in pairs])
        ks = np.stack([keys[m // H, :, m % H, :] for m in pairs])
        vs = np.stack([values[m // H, :, m % H, :] for m in pairs])
        in_maps.append(
            {
                "q": np.ascontiguousarray(qs),
                "k": np.ascontiguousarray(ks),
                "v": np.ascontiguousarray(vs),
                "maskneg": maskneg,
                "masknegT": masknegT,
                "ident": ident,
            }
        )
    return in_maps


def run(queries, keys, values, trace=False):
    _install_ntff_hook()
    from concourse.bass_utils import run_bass_kernel_spmd

    nc = build_nc()
    in_maps = make_in_maps(queries, keys, values)
    res = run_bass_kernel_spmd(
        nc, in_maps, core_ids=list(range(N_CORES)), trace=trace
    )
    V = np.empty((B, L, H, E), dtype=np.float32)
    SA = np.empty((B, H, L, L), dtype=np.float32)
    for c in range(N_CORES):
        out = res.results[c]
        for idx, m in enumerate([2 * c, 2 * c + 1]):
            b, h = m // H, m % H
            SA[b, h] = out["sa"][idx]
            V[b, :, h, :] = out["o"][idx]
    return (V, SA), res


def kernel(queries, keys, values):
    (V, SA), _ = run(queries, keys, values, trace=False)
    return (V, SA)


# revision 8
# speedup vs baseline: 1.4664x; 1.4664x over previous
"""Causal attention (B=2, L=2048, H=8, E=64) returning (V, SA) on 8 trn2 NeuronCores.

Sharding: the 16 (b,h) pairs are split 2-per-core (batch*head data parallel);
each core holds full L so the causal softmax needs no communication.

Per (b,h) on-chip plan (all matmuls bf16, accumulation f32):
  prep:   load Q,K,V f32 -> cast bf16; build Q^T,K^T [64,2048] via
          matmul-by-identity (PE) + PSUM->SBUF copy; V plus a ones column.
  phase1: S = Q K^T per 128-row tile -> additive causal mask on the diagonal
          block -> ACT exp(0.125*x) -> unnormalized E (bf16, SBUF).
  phase2: same matmuls with lhsT/rhs swapped -> E^T (bf16) directly
          (avoids any on-chip transpose of the 2048x2048 matrix).
  phase3: O_unnorm = E^T-blocks @ [V | 1]: the ones column accumulates the
          softmax row sums in PSUM col 64. reciprocal -> normalize SA rows
          (bf16*recip -> f32) and O, DMA out.
The strictly-upper causal triangle of SA is never written: PJRT donates
zero-initialized output buffers, so those regions read back as exact 0.
"""

import sys
import types

sys.path.insert(0, "/opt/trn_rl_repo")

import numpy as np
import ml_dtypes

B, L, H, E = 2, 2048, 8, 64
P = 128          # partitions / l-tile size
T = L // P       # 16 tiles
NBH = 2          # (b,h) pairs per core
N_CORES = 8
SCALE = 1.0 / 8.0
NEG = -1.0e9

_CACHE = {}


def _install_ntff_hook():
    """The image's antenv lacks axon_hooks; shim it so trace=True works."""
    import antenv

    if "antenv.axon_hooks" in sys.modules:
        return
    hooks = types.ModuleType("antenv.axon_hooks")
    hooks._hook = None
    hooks.set_axon_ntff_profile_hook = lambda h: setattr(hooks, "_hook", h)
    hooks.get_axon_ntff_profile_hook = lambda: hooks._hook
    sys.modules["antenv.axon_hooks"] = hooks
    antenv.axon_hooks = hooks
    try:
        from trn_agent_boot.trn_boot import _ntff_profile_via_ctypes

        hook = _ntff_profile_via_ctypes("/opt/axon/libaxon_pjrt.so")
        if hook is not None:
            hooks.set_axon_ntff_profile_hook(hook)
    except Exception:
        pass


def _emit(nc, tc, ctx, aps, mybir):
    import concourse.tile as tile  # noqa: F401

    f32 = mybir.dt.float32
    bf16 = mybir.dt.bfloat16
    Exp = mybir.ActivationFunctionType.Exp

    q = aps["q"].rearrange("n (t p) e -> n p t e", p=P)
    k = aps["k"].rearrange("n (t p) e -> n p t e", p=P)
    v = aps["v"].rearrange("n (t p) e -> n p t e", p=P)
    sa = aps["sa"].rearrange("n (t p) s -> n t p s", p=P)
    o = aps["o"].rearrange("n (t p) e -> n t p e", p=P)

    # packed-causal column offsets inside the E / E^T sbuf tiles
    offE = [P * (i * (i + 1) // 2) for i in range(T + 1)]            # row-tile i: width (i+1)*P
    offT = [P * (T * j - j * (j - 1) // 2) for j in range(T + 1)]    # col-tile j: width (T-j)*P
    CW = offE[T]  # 17408

    cpool = ctx.enter_context(tc.tile_pool(name="consts", bufs=1))
    ldpool = ctx.enter_context(tc.tile_pool(name="ld", bufs=3))
    bfpool = ctx.enter_context(tc.tile_pool(name="bfp", bufs=2))
    qktpool = ctx.enter_context(tc.tile_pool(name="qkt", bufs=2))
    bigpool = ctx.enter_context(tc.tile_pool(name="big", bufs=2))
    outpool = ctx.enter_context(tc.tile_pool(name="outp", bufs=4))
    smallpool = ctx.enter_context(tc.tile_pool(name="small", bufs=4))
    pspool = ctx.enter_context(tc.tile_pool(name="ps", bufs=2, space="PSUM"))
    popool = ctx.enter_context(tc.tile_pool(name="po", bufs=2, space="PSUM"))

    PSW = 1024  # S/ST psum tile width

    mask_sb = cpool.tile([P, P], f32, tag="mask")
    nc.sync.dma_start(mask_sb[:], aps["maskneg"][:])
    maskT_sb = cpool.tile([P, P], f32, tag="maskT")
    nc.sync.dma_start(maskT_sb[:], aps["masknegT"][:])
    ident_sb = cpool.tile([P, P], bf16, tag="ident")
    nc.sync.dma_start(ident_sb[:], aps["ident"][:])

    qt = {}
    kt = {}
    vb = {}
    et = {}
    recips = {}
    for bh in range(NBH):
        # ---- load + cast ----
        qf = ldpool.tile([P, T, E], f32, tag="ld")
        nc.sync.dma_start(qf[:], q[bh])
        kf = ldpool.tile([P, T, E], f32, tag="ld")
        nc.sync.dma_start(kf[:], k[bh])
        vf = ldpool.tile([P, T, E], f32, tag="ld")
        nc.sync.dma_start(vf[:], v[bh])

        qb = bfpool.tile([P, T, E], bf16, tag="qb")
        nc.vector.tensor_copy(qb[:], qf[:])
        kb = bfpool.tile([P, T, E], bf16, tag="kb")
        nc.vector.tensor_copy(kb[:], kf[:])
        vb[bh] = bfpool.tile([P, T, E], bf16, tag="vb", name=f"vb{bh}")
        nc.vector.tensor_copy(vb[bh][:], vf[:])

        # ---- Q^T / K^T via matmul-by-identity (one batched PSUM->SBUF cast) ----
        qt[bh] = qktpool.tile([E, L], bf16, tag="qt", name=f"qt{bh}")
        kt[bh] = qktpool.tile([E, L], bf16, tag="kt", name=f"kt{bh}")
        for src, dst in ((qb, qt[bh]), (kb, kt[bh])):
            pt = pspool.tile([E, PSW], f32, tag="ps")
            for t in range(8):
                nc.tensor.matmul(
                    pt[:, t * P : (t + 1) * P], src[:, t, :], ident_sb[:],
                    start=True, stop=True,
                )
            nc.vector.tensor_copy(dst[:, 0 : 8 * P], pt[:, 0 : 8 * P])
            pt2 = pspool.tile([E, PSW], f32, tag="ps")
            for t in range(8, T):
                tt = t - 8
                nc.tensor.matmul(
                    pt2[:, tt * P : (tt + 1) * P],
                    src[:, t, :], ident_sb[:], start=True, stop=True,
                )
            nc.vector.tensor_copy(dst[:, 8 * P : L], pt2[:, 0 : L - 8 * P])

        et[bh] = bigpool.tile([P, CW], bf16, tag="et", name=f"et{bh}")
        recips[bh] = smallpool.tile([P, T], f32, tag="recips", name=f"recips{bh}")

    # ---- phase 1 (both heads interleaved): S -> mask -> exp(+rowsum) ->
    #      reciprocal -> normalize in place -> DMA SA rows out ----
    for i in range(T):
        W = (i + 1) * P
        for bh in range(NBH):
            sums = smallpool.tile([P, 2], f32, tag="sums")
            sa_st = outpool.tile([P, 2048], f32, tag="sa")
            nparts = 0
            for p0 in range(0, W, PSW):
                pw = min(PSW, W - p0)
                ps = pspool.tile([P, PSW], f32, tag="ps")
                for c0 in range(0, pw, 512):
                    w = min(512, pw - c0)
                    nc.tensor.matmul(
                        ps[:, c0 : c0 + w],
                        qt[bh][:, i * P : (i + 1) * P],
                        kt[bh][:, p0 + c0 : p0 + c0 + w],
                        start=True,
                        stop=True,
                    )
                if p0 <= i * P < p0 + pw:  # diag block lives in this psum tile
                    d0 = i * P - p0
                    nc.vector.tensor_add(
                        ps[:, d0 : d0 + P], ps[:, d0 : d0 + P], mask_sb[:]
                    )
                nc.scalar.activation(
                    sa_st[:, p0 : p0 + pw], ps[:, 0:pw], Exp, scale=SCALE,
                    accum_out=sums[:, nparts : nparts + 1],
                )
                nparts += 1
            if nparts > 1:
                nc.vector.tensor_add(
                    sums[:, 0:1], sums[:, 0:1], sums[:, 1:2]
                )
            nc.vector.reciprocal(recips[bh][:, i : i + 1], sums[:, 0:1])
            nc.vector.tensor_scalar_mul(
                sa_st[:, 0:W], sa_st[:, 0:W], recips[bh][:, i : i + 1]
            )
            nc.sync.dma_start(sa[bh, i, :, 0:W], sa_st[:, 0:W])

    # ---- phase 2 (interleaved): E^T directly (swap lhsT/rhs) ----
    for j in range(T):
        Wj = (T - j) * P
        base = j * P
        for bh in range(NBH):
            first = True
            for p0 in range(0, Wj, PSW):
                pw = min(PSW, Wj - p0)
                ps = pspool.tile([P, PSW], f32, tag="ps")
                for c0 in range(0, pw, 512):
                    w = min(512, pw - c0)
                    nc.tensor.matmul(
                        ps[:, c0 : c0 + w],
                        kt[bh][:, base : base + P],
                        qt[bh][:, base + p0 + c0 : base + p0 + c0 + w],
                        start=True,
                        stop=True,
                    )
                if first:  # diag block is the first 128 cols
                    nc.vector.tensor_add(ps[:, 0:P], ps[:, 0:P], maskT_sb[:])
                    first = False
                nc.scalar.activation(
                    et[bh][:, offT[j] + p0 : offT[j] + p0 + pw],
                    ps[:, 0:pw], Exp, scale=SCALE,
                )

    # ---- phase 3 (interleaved): O = sum_j E^T_j-block @ V_j, normalize, store ----
    for i in range(T):
        for bh in range(NBH):
            po = popool.tile([P, E], f32, tag="po")
            for j in range(i + 1):
                nc.tensor.matmul(
                    po[:],
                    et[bh][:, offT[j] + (i - j) * P : offT[j] + (i - j + 1) * P],
                    vb[bh][:, j, :],
                    start=(j == 0),
                    stop=(j == i),
                )
            o_st = outpool.tile([P, E], f32, tag="o")
            nc.vector.tensor_scalar_mul(o_st[:], po[:], recips[bh][:, i : i + 1])
            nc.sync.dma_start(o[bh, i], o_st[:])


def build_nc():
    if "nc" in _CACHE:
        return _CACHE["nc"]
    from contextlib import ExitStack

    import concourse.bacc as bacc
    import concourse.mybir as mybir
    import concourse.tile as tile

    f32 = mybir.dt.float32
    bf16 = mybir.dt.bfloat16

    nc = bacc.Bacc("TRN2", target_bir_lowering=False, debug=False, num_devices=N_CORES)
    aps = {
        "q": nc.dram_tensor("q", [NBH, L, E], f32, kind="ExternalInput").ap(),
        "k": nc.dram_tensor("k", [NBH, L, E], f32, kind="ExternalInput").ap(),
        "v": nc.dram_tensor("v", [NBH, L, E], f32, kind="ExternalInput").ap(),
        "maskneg": nc.dram_tensor("maskneg", [P, P], f32, kind="ExternalInput").ap(),
        "masknegT": nc.dram_tensor("masknegT", [P, P], f32, kind="ExternalInput").ap(),
        "ident": nc.dram_tensor("ident", [P, P], bf16, kind="ExternalInput").ap(),
        "sa": nc.dram_tensor("sa", [NBH, L, L], f32, kind="ExternalOutput").ap(),
        "o": nc.dram_tensor("o", [NBH, L, E], f32, kind="ExternalOutput").ap(),
    }
    with tile.TileContext(nc) as tc, ExitStack() as ctx:
        _emit(nc, tc, ctx, aps, mybir)
    nc.compile()
    _CACHE["nc"] = nc
    return nc


def _host_consts():
    idx = np.arange(P)
    maskneg = np.where(idx[None, :] <= idx[:, None], 0.0, NEG).astype(np.float32)
    masknegT = maskneg.T.copy()
    ident = np.eye(P, dtype=ml_dtypes.bfloat16)
    return maskneg, masknegT, ident


def make_in_maps(queries, keys, values):
    queries = np.asarray(queries, dtype=np.float32)
    keys = np.asarray(keys, dtype=np.float32)
    values = np.asarray(values, dtype=np.float32)
    maskneg, masknegT, ident = _host_consts()
    in_maps = []
    for c in range(N_CORES):
        pairs = [2 * c, 2 * c + 1]
        qs = np.stack([queries[m // H, :, m % H, :] for m in pairs])
        ks = np.stack([keys[m // H, :, m % H, :] for m in pairs])
        vs = np.stack([values[m // H, :, m % H, :] for m in pairs])
        in_maps.append(
            {
                "q": np.ascontiguousarray(qs),
                "k": np.ascontiguousarray(ks),
                "v": np.ascontiguousarray(vs),
                "maskneg": maskneg,
                "masknegT": masknegT,
                "ident": ident,
            }
        )
    return in_maps


def run(queries, keys, values, trace=False):
    _install_ntff_hook()
    from concourse.bass_utils import run_bass_kernel_spmd

    nc = build_nc()
    in_maps = make_in_maps(queries, keys, values)
    res = run_bass_kernel_spmd(
        nc, in_maps, core_ids=list(range(N_CORES)), trace=trace
    )
    V = np.empty((B, L, H, E), dtype=np.float32)
    SA = np.empty((B, H, L, L), dtype=np.float32)
    for c in range(N_CORES):
        out = res.results[c]
        for idx, m in enumerate([2 * c, 2 * c + 1]):
            b, h = m // H, m % H
            SA[b, h] = out["sa"][idx]
            V[b, :, h, :] = out["o"][idx]
    return (V, SA), res


def kernel(queries, keys, values):
    (V, SA), _ = run(queries, keys, values, trace=False)
    return (V, SA)


# revision 10
# speedup vs baseline: 1.7019x; 1.1606x over previous
"""Causal attention (B=2, L=2048, H=8, E=64) returning (V, SA) on 8 trn2 NeuronCores.

Sharding: the 16 (b,h) pairs are split 2-per-core (batch*head data parallel);
each core holds full L so the causal softmax needs no communication.

Per (b,h) on-chip plan (all matmuls bf16, accumulation f32):
  prep:   load Q,K,V f32 -> cast bf16; build Q^T,K^T [64,2048] via
          matmul-by-identity (PE) + PSUM->SBUF copy; V plus a ones column.
  phase1: S = Q K^T per 128-row tile -> additive causal mask on the diagonal
          block -> ACT exp(0.125*x) -> unnormalized E (bf16, SBUF).
  phase2: same matmuls with lhsT/rhs swapped -> E^T (bf16) directly
          (avoids any on-chip transpose of the 2048x2048 matrix).
  phase3: O_unnorm = E^T-blocks @ [V | 1]: the ones column accumulates the
          softmax row sums in PSUM col 64. reciprocal -> normalize SA rows
          (bf16*recip -> f32) and O, DMA out.
The strictly-upper causal triangle of SA is never written: PJRT donates
zero-initialized output buffers, so those regions read back as exact 0.
"""

import sys
import types

sys.path.insert(0, "/opt/trn_rl_repo")

import numpy as np
import ml_dtypes

B, L, H, E = 2, 2048, 8, 64
P = 128          # partitions / l-tile size
T = L // P       # 16 tiles
NBH = 2          # (b,h) pairs per core
N_CORES = 8
SCALE = 1.0 / 8.0
NEG = -1.0e9

_CACHE = {}


def _install_ntff_hook():
    """The image's antenv lacks axon_hooks; shim it so trace=True works."""
    import antenv

    if "antenv.axon_hooks" in sys.modules:
        return
    hooks = types.ModuleType("antenv.axon_hooks")
    hooks._hook = None
    hooks.set_axon_ntff_profile_hook = lambda h: setattr(hooks, "_hook", h)
    hooks.get_axon_ntff_profile_hook = lambda: hooks._hook
    sys.modules["antenv.axon_hooks"] = hooks
    antenv.axon_hooks = hooks
    try:
        from trn_agent_boot.trn_boot import _ntff_profile_via_ctypes

        hook = _ntff_profile_via_ctypes("/opt/axon/libaxon_pjrt.so")
        if hook is not None:
            hooks.set_axon_ntff_profile_hook(hook)
    except Exception:
        pass


def _emit(nc, tc, ctx, aps, mybir):
    import concourse.tile as tile  # noqa: F401

    f32 = mybir.dt.float32
    bf16 = mybir.dt.bfloat16
    Exp = mybir.ActivationFunctionType.Exp

    q = aps["q"].rearrange("n (t p) e -> n p t e", p=P)
    k = aps["k"].rearrange("n (t p) e -> n p t e", p=P)
    v = aps["v"].rearrange("n (t p) e -> n p t e", p=P)
    sa = aps["sa"].rearrange("n (t p) s -> n t p s", p=P)
    o = aps["o"].rearrange("n (t p) e -> n t p e", p=P)

    # packed-causal column offsets inside the E / E^T sbuf tiles
    offE = [P * (i * (i + 1) // 2) for i in range(T + 1)]            # row-tile i: width (i+1)*P
    offT = [P * (T * j - j * (j - 1) // 2) for j in range(T + 1)]    # col-tile j: width (T-j)*P
    CW = offE[T]  # 17408

    cpool = ctx.enter_context(tc.tile_pool(name="consts", bufs=1))
    ldpool = ctx.enter_context(tc.tile_pool(name="ld", bufs=3))
    bfpool = ctx.enter_context(tc.tile_pool(name="bfp", bufs=2))
    qktpool = ctx.enter_context(tc.tile_pool(name="qkt", bufs=2))
    bigpool = ctx.enter_context(tc.tile_pool(name="big", bufs=2))
    outpool = ctx.enter_context(tc.tile_pool(name="outp", bufs=4))
    smallpool = ctx.enter_context(tc.tile_pool(name="small", bufs=4))
    pspool = ctx.enter_context(tc.tile_pool(name="ps", bufs=3, space="PSUM"))
    popool = ctx.enter_context(tc.tile_pool(name="po", bufs=2, space="PSUM"))

    PSW = 1024  # S/ST psum tile width

    mask_sb = cpool.tile([P, P], f32, tag="mask")
    nc.sync.dma_start(mask_sb[:], aps["maskneg"][:])
    maskT_sb = cpool.tile([P, P], f32, tag="maskT")
    nc.sync.dma_start(maskT_sb[:], aps["masknegT"][:])
    ident_sb = cpool.tile([P, P], bf16, tag="ident")
    nc.sync.dma_start(ident_sb[:], aps["ident"][:])

    qt = {}
    kt = {}
    vb = {}
    et = {}
    recips = {}
    for bh in range(NBH):
        # ---- load + cast ----
        qf = ldpool.tile([P, T, E], f32, tag="ld")
        nc.sync.dma_start(qf[:], q[bh])
        kf = ldpool.tile([P, T, E], f32, tag="ld")
        nc.sync.dma_start(kf[:], k[bh])
        vf = ldpool.tile([P, T, E], f32, tag="ld")
        nc.sync.dma_start(vf[:], v[bh])

        qb = bfpool.tile([P, T, E], bf16, tag="qb")
        nc.gpsimd.tensor_copy(qb[:], qf[:])
        kb = bfpool.tile([P, T, E], bf16, tag="kb")
        nc.gpsimd.tensor_copy(kb[:], kf[:])
        vb[bh] = bfpool.tile([P, T, E], bf16, tag="vb", name=f"vb{bh}")
        nc.gpsimd.tensor_copy(vb[bh][:], vf[:])

        # ---- Q^T / K^T via matmul-by-identity (one batched PSUM->SBUF cast) ----
        qt[bh] = qktpool.tile([E, L], bf16, tag="qt", name=f"qt{bh}")
        kt[bh] = qktpool.tile([E, L], bf16, tag="kt", name=f"kt{bh}")
        for src, dst in ((qb, qt[bh]), (kb, kt[bh])):
            pt = pspool.tile([E, PSW], f32, tag="ps")
            for t in range(8):
                nc.tensor.matmul(
                    pt[:, t * P : (t + 1) * P], src[:, t, :], ident_sb[:],
                    start=True, stop=True,
                )
            nc.vector.tensor_copy(dst[:, 0 : 8 * P], pt[:, 0 : 8 * P])
            pt2 = pspool.tile([E, PSW], f32, tag="ps")
            for t in range(8, T):
                tt = t - 8
                nc.tensor.matmul(
                    pt2[:, tt * P : (tt + 1) * P],
                    src[:, t, :], ident_sb[:], start=True, stop=True,
                )
            nc.vector.tensor_copy(dst[:, 8 * P : L], pt2[:, 0 : L - 8 * P])

        et[bh] = bigpool.tile([P, CW], bf16, tag="et", name=f"et{bh}")
        recips[bh] = smallpool.tile([P, T], f32, tag="recips", name=f"recips{bh}")

    # ---- fused main loop: per step i emit phase1(i), phase2(j=i), phase3(i)
    # for both heads; Tile's scheduler then overlaps PE matmul work (S, S^T, O)
    # with ACT exp and DVE normalize across steps ----
    for i in range(T):
        W = (i + 1) * P
        for bh in range(NBH):
            # phase 1: S row-tile i -> mask -> exp(+rowsum) -> normalize -> DMA
            sums = smallpool.tile([P, 2], f32, tag="sums")
            sa_st = outpool.tile([P, 2048], f32, tag="sa")
            nparts = 0
            for p0 in range(0, W, PSW):
                pw = min(PSW, W - p0)
                ps = pspool.tile([P, PSW], f32, tag="ps")
                for c0 in range(0, pw, 512):
                    w = min(512, pw - c0)
                    nc.tensor.matmul(
                        ps[:, c0 : c0 + w],
                        qt[bh][:, i * P : (i + 1) * P],
                        kt[bh][:, p0 + c0 : p0 + c0 + w],
                        start=True,
                        stop=True,
                    )
                if p0 <= i * P < p0 + pw:  # diag block lives in this psum tile
                    d0 = i * P - p0
                    nc.vector.tensor_add(
                        ps[:, d0 : d0 + P], ps[:, d0 : d0 + P], mask_sb[:]
                    )
                nc.scalar.activation(
                    sa_st[:, p0 : p0 + pw], ps[:, 0:pw], Exp, scale=SCALE,
                    accum_out=sums[:, nparts : nparts + 1],
                )
                nparts += 1
            if nparts > 1:
                nc.vector.tensor_add(sums[:, 0:1], sums[:, 0:1], sums[:, 1:2])
            nc.vector.reciprocal(recips[bh][:, i : i + 1], sums[:, 0:1])
            nc.vector.tensor_scalar_mul(
                sa_st[:, 0:W], sa_st[:, 0:W], recips[bh][:, i : i + 1]
            )
            nc.sync.dma_start(sa[bh, i, :, 0:W], sa_st[:, 0:W])

            # phase 2: E^T col-tile j=i (swap lhsT/rhs)
            j = i
            Wj = (T - j) * P
            base = j * P
            first = True
            for p0 in range(0, Wj, PSW):
                pw = min(PSW, Wj - p0)
                ps = pspool.tile([P, PSW], f32, tag="ps")
                for c0 in range(0, pw, 512):
                    w = min(512, pw - c0)
                    nc.tensor.matmul(
                        ps[:, c0 : c0 + w],
                        kt[bh][:, base : base + P],
                        qt[bh][:, base + p0 + c0 : base + p0 + c0 + w],
                        start=True,
                        stop=True,
                    )
                if first:  # diag block is the first 128 cols
                    nc.vector.tensor_add(ps[:, 0:P], ps[:, 0:P], maskT_sb[:])
                    first = False
                nc.scalar.activation(
                    et[bh][:, offT[j] + p0 : offT[j] + p0 + pw],
                    ps[:, 0:pw], Exp, scale=SCALE,
                )

            # phase 3: O row-tile i (needs E^T col-tiles 0..i, all emitted)
            po = popool.tile([P, E], f32, tag="po")
            for j3 in range(i + 1):
                nc.tensor.matmul(
                    po[:],
                    et[bh][:, offT[j3] + (i - j3) * P : offT[j3] + (i - j3 + 1) * P],
                    vb[bh][:, j3, :],
                    start=(j3 == 0),
                    stop=(j3 == i),
                )
            o_st = outpool.tile([P, E], f32, tag="o")
            nc.vector.tensor_scalar_mul(o_st[:], po[:], recips[bh][:, i : i + 1])
            nc.sync.dma_start(o[bh, i], o_st[:])


def build_nc():
    if "nc" in _CACHE:
        return _CACHE["nc"]
    from contextlib import ExitStack

    import concourse.bacc as bacc
    import concourse.mybir as mybir
    import concourse.tile as tile

    f32 = mybir.dt.float32
    bf16 = mybir.dt.bfloat16

    nc = bacc.Bacc("TRN2", target_bir_lowering=False, debug=False, num_devices=N_CORES)
    aps = {
        "q": nc.dram_tensor("q", [NBH, L, E], f32, kind="ExternalInput").ap(),
        "k": nc.dram_tensor("k", [NBH, L, E], f32, kind="ExternalInput").ap(),
        "v": nc.dram_tensor("v", [NBH, L, E], f32, kind="ExternalInput").ap(),
        "maskneg": nc.dram_tensor("maskneg", [P, P], f32, kind="ExternalInput").ap(),
        "masknegT": nc.dram_tensor("masknegT", [P, P], f32, kind="ExternalInput").ap(),
        "ident": nc.dram_tensor("ident", [P, P], bf16, kind="ExternalInput").ap(),
        "sa": nc.dram_tensor("sa", [NBH, L, L], f32, kind="ExternalOutput").ap(),
        "o": nc.dram_tensor("o", [NBH, L, E], f32, kind="ExternalOutput").ap(),
    }
    with tile.TileContext(nc) as tc, ExitStack() as ctx:
        _emit(nc, tc, ctx, aps, mybir)
    nc.compile()
    _CACHE["nc"] = nc
    return nc


def _host_consts():
    idx = np.arange(P)
    maskneg = np.where(idx[None, :] <= idx[:, None], 0.0, NEG).astype(np.float32)
    masknegT = maskneg.T.copy()
    ident = np.eye(P, dtype=ml_dtypes.bfloat16)
    return maskneg, masknegT, ident


def make_in_maps(queries, keys, values):
    queries = np.asarray(queries, dtype=np.float32)
    keys = np.asarray(keys, dtype=np.float32)
    values = np.asarray(values, dtype=np.float32)
    maskneg, masknegT, ident = _host_consts()
    in_maps = []
    for c in range(N_CORES):
        pairs = [2 * c, 2 * c + 1]
        qs = np.stack([queries[m // H, :, m % H, :] for m in pairs])
        ks = np.stack([keys[m // H, :, m % H, :] for m in pairs])
        vs = np.stack([values[m // H, :, m % H, :] for m in pairs])
        in_maps.append(
            {
                "q": np.ascontiguousarray(qs),
                "k": np.ascontiguousarray(ks),
                "v": np.ascontiguousarray(vs),
                "maskneg": maskneg,
                "masknegT": masknegT,
                "ident": ident,
            }
        )
    return in_maps


def run(queries, keys, values, trace=False):
    _install_ntff_hook()
    from concourse.bass_utils import run_bass_kernel_spmd

    nc = build_nc()
    in_maps = make_in_maps(queries, keys, values)
    res = run_bass_kernel_spmd(
        nc, in_maps, core_ids=list(range(N_CORES)), trace=trace
    )
    V = np.empty((B, L, H, E), dtype=np.float32)
    SA = np.empty((B, H, L, L), dtype=np.float32)
    for c in range(N_CORES):
        out = res.results[c]
        for idx, m in enumerate([2 * c, 2 * c + 1]):
            b, h = m // H, m % H
            SA[b, h] = out["sa"][idx]
            V[b, :, h, :] = out["o"][idx]
    return (V, SA), res


def kernel(queries, keys, values):
    (V, SA), _ = run(queries, keys, values, trace=False)
    return (V, SA)


# revision 11
# speedup vs baseline: 1.7456x; 1.0257x over previous
"""Causal attention (B=2, L=2048, H=8, E=64) returning (V, SA) on 8 trn2 NeuronCores.

Sharding: the 16 (b,h) pairs are split 2-per-core (batch*head data parallel);
each core holds full L so the causal softmax needs no communication.

Per (b,h) on-chip plan (all matmuls bf16, accumulation f32):
  prep:   load Q,K,V f32 -> cast bf16; build Q^T,K^T [64,2048] via
          matmul-by-identity (PE) + PSUM->SBUF copy; V plus a ones column.
  phase1: S = Q K^T per 128-row tile -> additive causal mask on the diagonal
          block -> ACT exp(0.125*x) -> unnormalized E (bf16, SBUF).
  phase2: same matmuls with lhsT/rhs swapped -> E^T (bf16) directly
          (avoids any on-chip transpose of the 2048x2048 matrix).
  phase3: O_unnorm = E^T-blocks @ [V | 1]: the ones column accumulates the
          softmax row sums in PSUM col 64. reciprocal -> normalize SA rows
          (bf16*recip -> f32) and O, DMA out.
The strictly-upper causal triangle of SA is never written: PJRT donates
zero-initialized output buffers, so those regions read back as exact 0.
"""

import sys
import types

sys.path.insert(0, "/opt/trn_rl_repo")

import numpy as np
import ml_dtypes

B, L, H, E = 2, 2048, 8, 64
P = 128          # partitions / l-tile size
T = L // P       # 16 tiles
NBH = 2          # (b,h) pairs per core
N_CORES = 8
SCALE = 1.0 / 8.0
NEG = -1.0e9

_CACHE = {}


def _install_ntff_hook():
    """The image's antenv lacks axon_hooks; shim it so trace=True works."""
    import antenv

    if "antenv.axon_hooks" in sys.modules:
        return
    hooks = types.ModuleType("antenv.axon_hooks")
    hooks._hook = None
    hooks.set_axon_ntff_profile_hook = lambda h: setattr(hooks, "_hook", h)
    hooks.get_axon_ntff_profile_hook = lambda: hooks._hook
    sys.modules["antenv.axon_hooks"] = hooks
    antenv.axon_hooks = hooks
    try:
        from trn_agent_boot.trn_boot import _ntff_profile_via_ctypes

        hook = _ntff_profile_via_ctypes("/opt/axon/libaxon_pjrt.so")
        if hook is not None:
            hooks.set_axon_ntff_profile_hook(hook)
    except Exception:
        pass


def _emit(nc, tc, ctx, aps, mybir):
    import concourse.tile as tile  # noqa: F401

    f32 = mybir.dt.float32
    bf16 = mybir.dt.bfloat16
    Exp = mybir.ActivationFunctionType.Exp

    q = aps["q"].rearrange("n (t p) e -> n p t e", p=P)
    k = aps["k"].rearrange("n (t p) e -> n p t e", p=P)
    v = aps["v"].rearrange("n (t p) e -> n p t e", p=P)
    sa = aps["sa"].rearrange("n (t p) s -> n t p s", p=P)
    o = aps["o"].rearrange("n (t p) e -> n t p e", p=P)

    # packed-causal column offsets inside the E / E^T sbuf tiles
    offE = [P * (i * (i + 1) // 2) for i in range(T + 1)]            # row-tile i: width (i+1)*P
    offT = [P * (T * j - j * (j - 1) // 2) for j in range(T + 1)]    # col-tile j: width (T-j)*P
    CW = offE[T]  # 17408

    cpool = ctx.enter_context(tc.tile_pool(name="consts", bufs=1))
    ldpool = ctx.enter_context(tc.tile_pool(name="ld", bufs=3))
    bfpool = ctx.enter_context(tc.tile_pool(name="bfp", bufs=2))
    qktpool = ctx.enter_context(tc.tile_pool(name="qkt", bufs=2))
    bigpool = ctx.enter_context(tc.tile_pool(name="big", bufs=2))
    outpool = ctx.enter_context(tc.tile_pool(name="outp", bufs=4))
    smallpool = ctx.enter_context(tc.tile_pool(name="small", bufs=4))
    pspool = ctx.enter_context(tc.tile_pool(name="ps", bufs=3, space="PSUM"))
    popool = ctx.enter_context(tc.tile_pool(name="po", bufs=2, space="PSUM"))

    PSW = 1024  # S/ST psum tile width

    mask_sb = cpool.tile([P, P], f32, tag="mask")
    nc.sync.dma_start(mask_sb[:], aps["maskneg"][:])
    maskT_sb = cpool.tile([P, P], f32, tag="maskT")
    nc.sync.dma_start(maskT_sb[:], aps["masknegT"][:])
    ident_sb = cpool.tile([P, P], bf16, tag="ident")
    nc.sync.dma_start(ident_sb[:], aps["ident"][:])

    qt = {}
    kt = {}
    vb = {}
    et = {}
    recips = {}
    for bh in range(NBH):
        # ---- load + cast ----
        qf = ldpool.tile([P, T, E], f32, tag="ld")
        nc.sync.dma_start(qf[:, 0:8, :], q[bh][:, 0:8, :])
        nc.sync.dma_start(qf[:, 8:T, :], q[bh][:, 8:T, :])
        kf = ldpool.tile([P, T, E], f32, tag="ld")
        nc.sync.dma_start(kf[:, 0:8, :], k[bh][:, 0:8, :])
        nc.sync.dma_start(kf[:, 8:T, :], k[bh][:, 8:T, :])
        vf = ldpool.tile([P, T, E], f32, tag="ld")
        nc.sync.dma_start(vf[:, 0:8, :], v[bh][:, 0:8, :])
        nc.sync.dma_start(vf[:, 8:T, :], v[bh][:, 8:T, :])

        qb = bfpool.tile([P, T, E], bf16, tag="qb")
        nc.vector.tensor_copy(qb[:], qf[:])
        kb = bfpool.tile([P, T, E], bf16, tag="kb")
        nc.vector.tensor_copy(kb[:], kf[:])
        vb[bh] = bfpool.tile([P, T, E], bf16, tag="vb", name=f"vb{bh}")
        nc.vector.tensor_copy(vb[bh][:], vf[:])

        # ---- Q^T / K^T via matmul-by-identity (one batched PSUM->SBUF cast) ----
        qt[bh] = qktpool.tile([E, L], bf16, tag="qt", name=f"qt{bh}")
        kt[bh] = qktpool.tile([E, L], bf16, tag="kt", name=f"kt{bh}")
        for src, dst in ((qb, qt[bh]), (kb, kt[bh])):
            pt = pspool.tile([E, PSW], f32, tag="ps")
            for t in range(8):
                nc.tensor.matmul(
                    pt[:, t * P : (t + 1) * P], src[:, t, :], ident_sb[:],
                    start=True, stop=True,
                )
            nc.vector.tensor_copy(dst[:, 0 : 8 * P], pt[:, 0 : 8 * P])
            pt2 = pspool.tile([E, PSW], f32, tag="ps")
            for t in range(8, T):
                tt = t - 8
                nc.tensor.matmul(
                    pt2[:, tt * P : (tt + 1) * P],
                    src[:, t, :], ident_sb[:], start=True, stop=True,
                )
            nc.vector.tensor_copy(dst[:, 8 * P : L], pt2[:, 0 : L - 8 * P])

        et[bh] = bigpool.tile([P, CW], bf16, tag="et", name=f"et{bh}")
        recips[bh] = smallpool.tile([P, T], f32, tag="recips", name=f"recips{bh}")

    # ---- fused main loop: per step i emit phase1(i), phase2(j=i), phase3(i)
    # for both heads; Tile's scheduler then overlaps PE matmul work (S, S^T, O)
    # with ACT exp and DVE normalize across steps ----
    for i in range(T):
        W = (i + 1) * P
        for bh in range(NBH):
            # phase 1: S row-tile i -> mask -> exp(+rowsum) -> normalize -> DMA
            sums = smallpool.tile([P, 2], f32, tag="sums")
            sa_st = outpool.tile([P, 2048], f32, tag="sa")
            nparts = 0
            for p0 in range(0, W, PSW):
                pw = min(PSW, W - p0)
                ps = pspool.tile([P, PSW], f32, tag="ps")
                for c0 in range(0, pw, 512):
                    w = min(512, pw - c0)
                    nc.tensor.matmul(
                        ps[:, c0 : c0 + w],
                        qt[bh][:, i * P : (i + 1) * P],
                        kt[bh][:, p0 + c0 : p0 + c0 + w],
                        start=True,
                        stop=True,
                    )
                if p0 <= i * P < p0 + pw:  # diag block lives in this psum tile
                    d0 = i * P - p0
                    nc.vector.tensor_add(
                        ps[:, d0 : d0 + P], ps[:, d0 : d0 + P], mask_sb[:]
                    )
                nc.scalar.activation(
                    sa_st[:, p0 : p0 + pw], ps[:, 0:pw], Exp, scale=SCALE,
                    accum_out=sums[:, nparts : nparts + 1],
                )
                nparts += 1
            if nparts > 1:
                nc.vector.tensor_add(sums[:, 0:1], sums[:, 0:1], sums[:, 1:2])
            nc.vector.reciprocal(recips[bh][:, i : i + 1], sums[:, 0:1])
            nc.vector.tensor_scalar_mul(
                sa_st[:, 0:W], sa_st[:, 0:W], recips[bh][:, i : i + 1]
            )
            nc.sync.dma_start(sa[bh, i, :, 0:W], sa_st[:, 0:W])

            # phase 2: E^T col-tile j=i (swap lhsT/rhs)
            j = i
            Wj = (T - j) * P
            base = j * P
            first = True
            for p0 in range(0, Wj, PSW):
                pw = min(PSW, Wj - p0)
                ps = pspool.tile([P, PSW], f32, tag="ps")
                for c0 in range(0, pw, 512):
                    w = min(512, pw - c0)
                    nc.tensor.matmul(
                        ps[:, c0 : c0 + w],
                        kt[bh][:, base : base + P],
                        qt[bh][:, base + p0 + c0 : base + p0 + c0 + w],
                        start=True,
                        stop=True,
                    )
                if first:  # diag block is the first 128 cols
                    nc.vector.tensor_add(ps[:, 0:P], ps[:, 0:P], maskT_sb[:])
                    first = False
                nc.scalar.activation(
                    et[bh][:, offT[j] + p0 : offT[j] + p0 + pw],
                    ps[:, 0:pw], Exp, scale=SCALE,
                )

            # phase 3: O row-tile i (needs E^T col-tiles 0..i, all emitted)
            po = popool.tile([P, E], f32, tag="po")
            for j3 in range(i + 1):
                nc.tensor.matmul(
                    po[:],
                    et[bh][:, offT[j3] + (i - j3) * P : offT[j3] + (i - j3 + 1) * P],
                    vb[bh][:, j3, :],
                    start=(j3 == 0),
                    stop=(j3 == i),
                )
            o_st = outpool.tile([P, E], f32, tag="o")
            nc.vector.tensor_scalar_mul(o_st[:], po[:], recips[bh][:, i : i + 1])
            nc.sync.dma_start(o[bh, i], o_st[:])


def build_nc():
    if "nc" in _CACHE:
        return _CACHE["nc"]
    from contextlib import ExitStack

    import concourse.bacc as bacc
    import concourse.mybir as mybir
    import concourse.tile as tile

    f32 = mybir.dt.float32
    bf16 = mybir.dt.bfloat16

    nc = bacc.Bacc("TRN2", target_bir_lowering=False, debug=False, num_devices=N_CORES)
    aps = {
        "q": nc.dram_tensor("q", [NBH, L, E], f32, kind="ExternalInput").ap(),
        "k": nc.dram_tensor("k", [NBH, L, E], f32, kind="ExternalInput").ap(),
        "v": nc.dram_tensor("v", [NBH, L, E], f32, kind="ExternalInput").ap(),
        "maskneg": nc.dram_tensor("maskneg", [P, P], f32, kind="ExternalInput").ap(),
        "masknegT": nc.dram_tensor("masknegT", [P, P], f32, kind="ExternalInput").ap(),
        "ident": nc.dram_tensor("ident", [P, P], bf16, kind="ExternalInput").ap(),
        "sa": nc.dram_tensor("sa", [NBH, L, L], f32, kind="ExternalOutput").ap(),
        "o": nc.dram_tensor("o", [NBH, L, E], f32, kind="ExternalOutput").ap(),
    }
    with tile.TileContext(nc) as tc, ExitStack() as ctx:
        _emit(nc, tc, ctx, aps, mybir)
    nc.compile()
    _CACHE["nc"] = nc
    return nc


def _host_consts():
    idx = np.arange(P)
    maskneg = np.where(idx[None, :] <= idx[:, None], 0.0, NEG).astype(np.float32)
    masknegT = maskneg.T.copy()
    ident = np.eye(P, dtype=ml_dtypes.bfloat16)
    return maskneg, masknegT, ident


def make_in_maps(queries, keys, values):
    queries = np.asarray(queries, dtype=np.float32)
    keys = np.asarray(keys, dtype=np.float32)
    values = np.asarray(values, dtype=np.float32)
    maskneg, masknegT, ident = _host_consts()
    in_maps = []
    for c in range(N_CORES):
        pairs = [2 * c, 2 * c + 1]
        qs = np.stack([queries[m // H, :, m % H, :] for m in pairs])
        ks = np.stack([keys[m // H, :, m % H, :] for m in pairs])
        vs = np.stack([values[m // H, :, m % H, :] for m in pairs])
        in_maps.append(
            {
                "q": np.ascontiguousarray(qs),
                "k": np.ascontiguousarray(ks),
                "v": np.ascontiguousarray(vs),
                "maskneg": maskneg,
                "masknegT": masknegT,
                "ident": ident,
            }
        )
    return in_maps


def run(queries, keys, values, trace=False):
    _install_ntff_hook()
    from concourse.bass_utils import run_bass_kernel_spmd

    nc = build_nc()
    in_maps = make_in_maps(queries, keys, values)
    res = run_bass_kernel_spmd(
        nc, in_maps, core_ids=list(range(N_CORES)), trace=trace
    )
    V = np.empty((B, L, H, E), dtype=np.float32)
    SA = np.empty((B, H, L, L), dtype=np.float32)
    for c in range(N_CORES):
        out = res.results[c]
        for idx, m in enumerate([2 * c, 2 * c + 1]):
            b, h = m // H, m % H
            SA[b, h] = out["sa"][idx]
            V[b, :, h, :] = out["o"][idx]
    return (V, SA), res


def kernel(queries, keys, values):
    (V, SA), _ = run(queries, keys, values, trace=False)
    return (V, SA)
